# revision 12
# baseline (speedup 1.0000x reference)
"""Sharded exact-kNN (DkNN) kernel for 8 Trainium2 NeuronCores.

Math: rows are L2-normalized, then centered by the mean of normalized train
rows. The same center is subtracted from query and train vectors, so
||qc - xc||^2 == ||qn - xn||^2 = 2 - 2*qn.xn (unit vectors): exact kNN is a
max-inner-product search over normalized vectors; the center cancels.

Device (per core, train table sharded 8 ways along N):
  bf16 matmul S = qn.T @ xn_shard (fp32 PSUM accumulation), then a ScalarE
  copy quantizes scores to uint8 (s*127.5 + 127.5, truncated) and the full
  [1024, 12544] uint8 score matrix is DMA'd out. That's the whole kernel:
  GEMM + quantize + dump.

Host: exact selection. Per query, find the 75th-largest uint8 level across
all 100k quantized scores, recompute EXACT fp32 scores for the ~200 elements
within a 3-level band of it (covers uint8 truncation + worst-case bf16
rounding, Cauchy-Schwarz bound 2^-7), take the exact top-75 from the band,
then reference-style distances and label counts. An a-posteriori per-row
check (quantized upper bound of all out-of-band elements + error bound <
exact 75th value) falls back to a full BLAS row recompute if the band could
have missed anything, making the result exact regardless of rounding.
"""
import sys
import numpy as np

sys.path.insert(0, "/opt/trn_rl_repo")

import concourse.bacc as bacc  # noqa: E402
import concourse.mybir as mybir  # noqa: E402
from concourse.tile import TileContext  # noqa: E402
from concourse.bass_utils import run_bass_kernel_spmd  # noqa: E402

import ml_dtypes  # noqa: E402

BF16 = ml_dtypes.bfloat16

# problem shapes (hardcoded per contest contract)
B = 1024          # queries
D = 256           # feature dim
N = 100000        # train rows
K = 75            # neighbors
NB_CLASSES = 10
NCORES = 8

NSH = 12500       # shard size
NS = 12544        # padded shard size (28 * 448)
BLK = 448         # matmul free-dim block == one PSUM bank (448*4B <= 2KB)
NBLK = NS // BLK  # 28
NCH = B // 128    # 8 query chunks
QSCALE = 127.5    # uint8 code = trunc(s*QSCALE + QSCALE), s in (-1, 1)
TH = 3            # band half-width in uint8 levels for exact host recompute
ERR = 2.0 ** -7   # rigorous |s_true - s_device| bound for bf16 inputs (C-S)

f32 = mybir.dt.float32
bf16 = mybir.dt.bfloat16
u8 = mybir.dt.uint8

_COMPILED = {}


def build(rep: int = 1):
    """Build + compile the per-core bass program. rep>1 repeats the body for
    on-hardware timing by differencing."""
    nc = bacc.Bacc("TRN2", target_bir_lowering=False, debug=False,
                   num_devices=NCORES)
    qt_in = nc.declare_dram_parameter("qt", [2, 128, B], bf16, isOutput=False)
    xt_in = nc.declare_dram_parameter("xt", [2, 128, NS], bf16, isOutput=False)
    y_s8 = nc.declare_dram_parameter("y_s8", [B, NS], u8, isOutput=True)

    with TileContext(nc) as tc:
        with (
            tc.tile_pool(name="inp", bufs=1) as inp,
            tc.tile_pool(name="stage", bufs=2) as stagep,
            tc.tile_pool(name="ps", bufs=8, space="PSUM") as psp,
        ):
            qts, xts = [], []
            for k in range(2):
                xtk = inp.tile([128, NS], bf16, tag=f"xt{k}")
                nc.sync.dma_start(out=xtk[:], in_=xt_in[k])
                xts.append(xtk)
                qtk = inp.tile([128, B], bf16, tag=f"qt{k}")
                nc.sync.dma_start(out=qtk[:], in_=qt_in[k])
                qts.append(qtk)

            for _ in range(rep):
                for c in range(NCH):
                    st = stagep.tile([128, NS], u8, tag="st")
                    for b in range(NBLK):
                        ps = psp.tile([128, BLK], f32, tag="ps")
                        col0 = b * BLK
                        for k in range(2):
                            nc.tensor.matmul(
                                ps[:],
                                lhsT=qts[k][:, c * 128:(c + 1) * 128],
                                rhs=xts[k][:, col0:col0 + BLK],
                                start=(k == 0), stop=(k == 1),
                            )
                        nc.scalar.activation(
                            st[:, col0:col0 + BLK], ps[:],
                            mybir.ActivationFunctionType.Copy,
                            bias=QSCALE, scale=QSCALE,
                        )
                    nc.sync.dma_start(
                        out=y_s8[c * 128:(c + 1) * 128, :], in_=st[:])

    nc.compile()
    return nc


def _get_compiled(rep: int = 1):
    if rep not in _COMPILED:
        _COMPILED[rep] = build(rep)
    return _COMPILED[rep]


def _prep_inputs(queries, train_feats):
    """Host prep: normalize rows, transpose to [D, N] bf16, shard, pad."""
    q = np.asarray(queries, np.float32)
    x = np.asarray(train_feats, np.float32)
    qn = q / np.linalg.norm(q, axis=1, keepdims=True)
    xn = x / np.linalg.norm(x, axis=1, keepdims=True)
    qt = np.ascontiguousarray(qn.T).astype(BF16).reshape(2, 128, B)
    in_maps = []
    for c in range(NCORES):
        xt = np.zeros((D, NS), BF16)
        xt[:, :NSH] = xn[c * NSH:(c + 1) * NSH].T.astype(BF16)
        in_maps.append({"qt": qt, "xt": xt.reshape(2, 128, NS)})
    return in_maps, qn, xn


def _merge(results, qn, xn, train_labels):
    """Host: exact top-K selection from the quantized score dump."""
    s8 = np.empty((B, N), np.uint8)
    for c in range(NCORES):
        s8[:, c * NSH:(c + 1) * NSH] = results[c]["y_s8"][:, :NSH]

    # per-row 75th-largest uint8 level via histogram
    hist = np.zeros((B, 256), np.int64)
    for b in range(B):
        hist[b] = np.bincount(s8[b], minlength=256)
    cum = np.cumsum(hist[:, ::-1], axis=1)[:, ::-1]   # cum[b,q] = #(s8 >= q)
    q75 = 255 - np.argmax((cum >= K)[:, ::-1], axis=1)

    lo = np.maximum(q75 - TH, 0).astype(np.uint8)
    mask = s8 >= lo[:, None]
    b_arr, n_arr = np.nonzero(mask)

    # exact fp32 scores for the band
    s_exact = np.einsum('ij,ij->i', qn[b_arr], xn[n_arr]).astype(np.float32)

    # exact top-K within each row's band
    order = np.lexsort((n_arr, -s_exact, b_arr))
    bs, ss, ns = b_arr[order], s_exact[order], n_arr[order]
    starts = np.searchsorted(bs, np.arange(B))
    ranks = np.arange(len(bs)) - starts[bs]
    sel = ranks < K
    cand_vals = ss[sel].reshape(B, K)
    cand_idx = ns[sel].reshape(B, K)

    # a-posteriori exactness check: any element outside the band satisfies
    # s_true <= ub(lo - 1) + ERR; must be below the row's exact K-th value.
    v75 = cand_vals[:, K - 1]
    ub_outside = (lo.astype(np.float32) - QSCALE) / QSCALE + ERR
    bad_rows = np.where(ub_outside >= v75)[0]
    for b in bad_rows:
        s_full = (xn @ qn[b]).astype(np.float32)
        p = np.argpartition(-s_full, K - 1)[:K]
        o = np.lexsort((p, -s_full[p]))
        cand_vals[b] = s_full[p][o]
        cand_idx[b] = p[o]

    # reference-style distances, ordered like top_k(-d2) (ties: lower index)
    center = xn.mean(axis=0, dtype=np.float32)
    qc = qn - center
    xc_sel = xn[cand_idx] - center                       # [B, K, D]
    d2 = (np.sum(qc * qc, axis=1)[:, None]
          + np.einsum('bkd,bkd->bk', xc_sel, xc_sel)
          - 2.0 * np.einsum('bd,bkd->bk', qc, xc_sel)).astype(np.float32)
    order = np.lexsort((cand_idx, d2), axis=1)
    d2_o = np.take_along_axis(d2, order, axis=1)
    idx_o = np.take_along_axis(cand_idx, order, axis=1).astype(np.int32)

    topk_dists = np.sqrt(np.maximum(d2_o, 0.0)).astype(np.float32)
    labels = np.asarray(train_labels)
    nb_labels = labels[idx_o]
    class_counts = np.zeros((B, NB_CLASSES), np.float32)
    for cc in range(NB_CLASSES):
        class_counts[:, cc] = (nb_labels == cc).sum(axis=1)
    return topk_dists, idx_o, class_counts


def kernel(queries, train_feats, train_labels):
    nc = _get_compiled()
    in_maps, qn, xn = _prep_inputs(queries, train_feats)
    res = run_bass_kernel_spmd(nc, in_maps, list(range(NCORES)))
    return _merge(res.results, qn, xn, train_labels)


# revision 18
# speedup vs baseline: 920.3935x; 920.3935x over previous
"""Sharded exact-kNN (DkNN) kernel for 8 Trainium2 NeuronCores.

Math: rows are L2-normalized, then centered by the mean of normalized train
rows. The same center is subtracted from query and train vectors, so
||qc - xc||^2 == ||qn - xn||^2 = 2 - 2*qn.xn (unit vectors): exact kNN is a
max-inner-product search over normalized vectors; the center cancels.

Device (per core, train table sharded 8 ways along N):
  bf16 matmul S = qn.T @ xn_shard (fp32 PSUM accumulation), then a ScalarE
  copy quantizes scores to uint8 (s*127.5 + 127.5, truncated) and the full
  [1024, 12544] uint8 score matrix is DMA'd out. That's the whole kernel:
  GEMM + quantize + dump.

Host: exact selection. Per query, find the 75th-largest uint8 level across
all 100k quantized scores, recompute EXACT fp32 scores for the ~200 elements
within a 3-level band of it (covers uint8 truncation + worst-case bf16
rounding, Cauchy-Schwarz bound 2^-7), take the exact top-75 from the band,
then reference-style distances and label counts. An a-posteriori per-row
check (quantized upper bound of all out-of-band elements + error bound <
exact 75th value) falls back to a full BLAS row recompute if the band could
have missed anything, making the result exact regardless of rounding.
"""
import sys
import numpy as np

sys.path.insert(0, "/opt/trn_rl_repo")

import concourse.bacc as bacc  # noqa: E402
import concourse.mybir as mybir  # noqa: E402
from concourse.tile import TileContext  # noqa: E402
from concourse.bass_utils import run_bass_kernel_spmd  # noqa: E402

import ml_dtypes  # noqa: E402

BF16 = ml_dtypes.bfloat16

# problem shapes (hardcoded per contest contract)
B = 1024          # queries
D = 256           # feature dim
N = 100000        # train rows
K = 75            # neighbors
NB_CLASSES = 10
NCORES = 8

NSH = 12500       # shard size
NS = 12544        # padded shard size (28 * 448)
BLK = 448         # matmul free-dim block == one PSUM bank (448*4B <= 2KB)
NBLK = NS // BLK  # 28
NCH = B // 128    # 8 query chunks
QSCALE = 127.5    # uint8 code = trunc(s*QSCALE + QSCALE), s in (-1, 1)
TH = 3            # band half-width in uint8 levels for exact host recompute
ERR = 2.0 ** -7   # rigorous |s_true - s_device| bound for bf16 inputs (C-S)

f32 = mybir.dt.float32
bf16 = mybir.dt.bfloat16
u8 = mybir.dt.uint8

_COMPILED = {}


def build(rep: int = 1):
    """Build + compile the per-core bass program. rep>1 repeats the body for
    on-hardware timing by differencing."""
    nc = bacc.Bacc("TRN2", target_bir_lowering=False, debug=False,
                   num_devices=NCORES)
    qt_in = nc.declare_dram_parameter("qt", [2, 128, B], bf16, isOutput=False)
    xt_in = nc.declare_dram_parameter("xt", [2, 128, NS], bf16, isOutput=False)
    y_s8 = nc.declare_dram_parameter("y_s8", [B, NS], u8, isOutput=True)

    with TileContext(nc) as tc:
        with (
            tc.tile_pool(name="inp", bufs=1) as inp,
            tc.tile_pool(name="stage", bufs=2) as stagep,
            tc.tile_pool(name="ps", bufs=8, space="PSUM") as psp,
        ):
            qts, xts = [], []
            for k in range(2):
                qtk = inp.tile([128, B], bf16, tag=f"qt{k}")
                nc.sync.dma_start(out=qtk[:], in_=qt_in[k])
                qts.append(qtk)
            xt0 = inp.tile([128, NS], bf16, tag="xt0")
            xt1 = inp.tile([128, NS], bf16, tag="xt1")
            xts = [xt0, xt1]
            bounds = [0, 1024, 3136, 6272, 9408, NS]
            for lo, hi in zip(bounds[:-1], bounds[1:]):
                for k in range(2):
                    nc.sync.dma_start(out=xts[k][:, lo:hi],
                                      in_=xt_in[k, :, lo:hi])

            # 24 blocks of 512 + one 256 tail
            cols = [(i * 512, 512) for i in range(24)] + [(24 * 512, 256)]
            for _ in range(rep):
                for c in range(NCH):
                    st = stagep.tile([128, NS], u8, tag="st")
                    for bi, (col0, w) in enumerate(cols):
                        ps = psp.tile([128, 512], f32, tag="ps")
                        for k in range(2):
                            nc.tensor.matmul(
                                ps[:, :w],
                                lhsT=qts[k][:, c * 128:(c + 1) * 128],
                                rhs=xts[k][:, col0:col0 + w],
                                start=(k == 0), stop=(k == 1),
                            )
                        # quantize PSUM -> uint8; alternate ScalarE/VectorE so
                        # neither copy engine gates PSUM bank recycling
                        if bi % 2 == 1:
                            nc.vector.tensor_scalar(
                                st[:, col0:col0 + w], ps[:, :w],
                                QSCALE, QSCALE,
                                op0=mybir.AluOpType.mult,
                                op1=mybir.AluOpType.add)
                        else:
                            nc.scalar.activation(
                                st[:, col0:col0 + w], ps[:, :w],
                                mybir.ActivationFunctionType.Copy,
                                bias=QSCALE, scale=QSCALE,
                            )
                    nc.sync.dma_start(
                        out=y_s8[c * 128:(c + 1) * 128, :], in_=st[:])

    nc.compile()
    return nc


def _get_compiled(rep: int = 1):
    if rep not in _COMPILED:
        _COMPILED[rep] = build(rep)
    return _COMPILED[rep]


def _prep_inputs(queries, train_feats):
    """Host prep: normalize rows, transpose to [D, N] bf16, shard, pad."""
    q = np.asarray(queries, np.float32)
    x = np.asarray(train_feats, np.float32)
    qn = q / np.linalg.norm(q, axis=1, keepdims=True)
    xn = x / np.linalg.norm(x, axis=1, keepdims=True)
    qt = np.ascontiguousarray(qn.T).astype(BF16).reshape(2, 128, B)
    in_maps = []
    for c in range(NCORES):
        xt = np.zeros((D, NS), BF16)
        xt[:, :NSH] = xn[c * NSH:(c + 1) * NSH].T.astype(BF16)
        in_maps.append({"qt": qt, "xt": xt.reshape(2, 128, NS)})
    return in_maps, qn, xn


def _merge(results, qn, xn, train_labels):
    """Host: exact top-K selection from the quantized score dump."""
    s8 = np.empty((B, N), np.uint8)
    for c in range(NCORES):
        s8[:, c * NSH:(c + 1) * NSH] = results[c]["y_s8"][:, :NSH]

    # per-row 75th-largest uint8 level via histogram
    hist = np.zeros((B, 256), np.int64)
    for b in range(B):
        hist[b] = np.bincount(s8[b], minlength=256)
    cum = np.cumsum(hist[:, ::-1], axis=1)[:, ::-1]   # cum[b,q] = #(s8 >= q)
    q75 = 255 - np.argmax((cum >= K)[:, ::-1], axis=1)

    lo = np.maximum(q75 - TH, 0).astype(np.uint8)
    mask = s8 >= lo[:, None]
    b_arr, n_arr = np.nonzero(mask)

    # exact fp32 scores for the band
    s_exact = np.einsum('ij,ij->i', qn[b_arr], xn[n_arr]).astype(np.float32)

    # exact top-K within each row's band
    order = np.lexsort((n_arr, -s_exact, b_arr))
    bs, ss, ns = b_arr[order], s_exact[order], n_arr[order]
    starts = np.searchsorted(bs, np.arange(B))
    ranks = np.arange(len(bs)) - starts[bs]
    sel = ranks < K
    cand_vals = ss[sel].reshape(B, K)
    cand_idx = ns[sel].reshape(B, K)

    # a-posteriori exactness check: any element outside the band satisfies
    # s_true <= ub(lo - 1) + ERR; must be below the row's exact K-th value.
    v75 = cand_vals[:, K - 1]
    ub_outside = (lo.astype(np.float32) - QSCALE) / QSCALE + ERR
    bad_rows = np.where(ub_outside >= v75)[0]
    for b in bad_rows:
        s_full = (xn @ qn[b]).astype(np.float32)
        p = np.argpartition(-s_full, K - 1)[:K]
        o = np.lexsort((p, -s_full[p]))
        cand_vals[b] = s_full[p][o]
        cand_idx[b] = p[o]

    # reference-style distances, ordered like top_k(-d2) (ties: lower index)
    center = xn.mean(axis=0, dtype=np.float32)
    qc = qn - center
    xc_sel = xn[cand_idx] - center                       # [B, K, D]
    d2 = (np.sum(qc * qc, axis=1)[:, None]
          + np.einsum('bkd,bkd->bk', xc_sel, xc_sel)
          - 2.0 * np.einsum('bd,bkd->bk', qc, xc_sel)).astype(np.float32)
    order = np.lexsort((cand_idx, d2), axis=1)
    d2_o = np.take_along_axis(d2, order, axis=1)
    idx_o = np.take_along_axis(cand_idx, order, axis=1).astype(np.int32)

    topk_dists = np.sqrt(np.maximum(d2_o, 0.0)).astype(np.float32)
    labels = np.asarray(train_labels)
    nb_labels = labels[idx_o]
    class_counts = np.zeros((B, NB_CLASSES), np.float32)
    for cc in range(NB_CLASSES):
        class_counts[:, cc] = (nb_labels == cc).sum(axis=1)
    return topk_dists, idx_o, class_counts


def kernel(queries, train_feats, train_labels):
    nc = _get_compiled()
    in_maps, qn, xn = _prep_inputs(queries, train_feats)
    res = run_bass_kernel_spmd(nc, in_maps, list(range(NCORES)))
    return _merge(res.results, qn, xn, train_labels)
